# revision 23
# baseline (speedup 1.0000x reference)
"""Cross-attention kernel for 8 TRN2 NeuronCores (Bass/Tile).

Reference (fp32):
    q = x @ Wq; k = ctx @ Wk; v = ctx @ Wv        (8 heads, d=64)
    sim = q k^T * d^-0.5 ; attn = softmax(sim)
    out = (attn v) @ Wo + bo

Sharding (data-parallel, no FLOP duplication): core c -> batch c//2,
head-group c%2 (4 heads).  Each core computes a partial [2048, 1024]
output; the host sums the two partials per batch and adds bo.

Per-core dataflow (bf16 matmul operands, fp32 accumulation):
  - host pre-shuffles every input into its exact SBUF image
    ([128, fat-contiguous-run] per partition) for full-rate DMA; x is
    i-chunk-major, ctx is j-half-major (so V projections can start on
    the first half while the second streams in)
  - input DMA is split across the sync and gpsimd queues in critical-
    path order: {wv, ctx, wk, x2, x3, wo} on sync, {wq, x0, x1} on
    gpsimd; V -> KT -> QT projections chase the DMA so the PE is doing
    useful work from ~7us (a short junk-matmul burst first trips the
    HAM clock-gate out of its cold 0.65-1.2 GHz p-state)
  - QT[d,i] = Wq^T x^T ; KT[d,j] = Wk^T ctx^T ; V[j,d] = ctx Wv, with a
    ones column appended per head ([V_h | 1], memset on-device)
  - simT[j,i] = KT_h-slice @ QT_h (K=64; head pairs land on PE row
    groups 0-63/64-127), fp32 PSUM
  - expT = exp(0.125 simT) on ScalarE (unsafe softmax, |sim|*0.125 < 4)
  - av = [V_h|1]^T @ expT accumulated over j -> [65, i] fp32 PSUM;
    row 64 is the softmax denominator
  - denominator: transpose to [128, 8] via DRAM (reciprocal is
    ~7 cyc/elem/lane), reciprocal, broadcast over 64 partitions via a
    DRAM round-trip; out2T = av[0:64] * recip  -> exactly the lhsT
    layout the Wo matmul needs
  - out = out2T^T @ Wo: half0 mid-attention (PE slack); half1's m=0..3
    t0-partials run during the last norm's DMA-latency window (their
    lhsT half is ready one group earlier), so the PE never idles long
    enough for HAM to re-throttle; psum evacuation is DVE mid-pipeline
    (ScalarE is exp-saturated) and ACT/DVE alternating in the tail;
    output staged bf16 (host accumulates in fp32)
The kernel is PE-stream-bound (~221k matmul columns/core at 2.4 GHz);
ScalarE (64 exps over 8.4M elements) runs just under it.
"""

import numpy as np
import ml_dtypes

import concourse.bass as bass
import concourse.tile as tile
from concourse import bacc, mybir
from concourse.bass_utils import run_bass_kernel_spmd

B = 4
I = 2048
J = 1024
FQ = 1024
FC = 768
DH = 64
HPC = 4
DG = HPC * DH      # 256
E = 1024
P = 128
N_CORES = 8
IH = I // 2        # 1024

F32 = mybir.dt.float32
BF16 = mybir.dt.bfloat16

KQ = FQ // P       # 8
KC = FC // P       # 6
TD = DG // P       # 2
JBN = J // P       # 8
ICN = 4            # x i-chunks


def _build():
    nc = bacc.Bacc()
    xt = nc.declare_dram_parameter("xt", [P, KQ * I], BF16, isOutput=False)
    ctxt = nc.declare_dram_parameter("ctxt", [P, KC * J], BF16, isOutput=False)
    wq = nc.declare_dram_parameter("wq", [P, KQ * DG], BF16, isOutput=False)
    wk = nc.declare_dram_parameter("wk", [P, KC * DG], BF16, isOutput=False)
    wv = nc.declare_dram_parameter("wv", [P, KC * DG], BF16, isOutput=False)
    wo = nc.declare_dram_parameter("wo", [P, TD * E], BF16, isOutput=False)
    out = nc.declare_dram_parameter("out", [I, E], BF16, isOutput=True)
    brc = nc.dram_tensor("brc", [2 * HPC, IH], F32)
    brc2 = nc.dram_tensor("brc2", [2 * HPC, IH], F32)

    with tile.TileContext(nc) as tc:
        with (
            tc.tile_pool(name="consts", bufs=1) as consts,
            tc.tile_pool(name="expp", bufs=40) as expp,
            tc.tile_pool(name="misc", bufs=3) as misc,
            tc.tile_pool(name="outp", bufs=3) as outp,
            tc.tile_pool(name="pp", bufs=2, space="PSUM") as pp,
            tc.tile_pool(name="pp2", bufs=2, space="PSUM") as pp2,
            tc.tile_pool(name="avp", bufs=1, space="PSUM") as avpool,
        ):
            # ---- PE warm-up: junk matmuls trip the HAM clock-gate (cold
            # PE runs at a low p-state) while the first loads stream in
            junk = consts.tile([P, P], BF16, tag="junk")
            nc.vector.memset(junk, 0.0)
            jps = pp2.tile([P, 512], F32, tag="pp2", name="jps")
            for w in range(58):
                nc.tensor.matmul(jps[:, :P], lhsT=junk, rhs=junk,
                                 start=True, stop=True)

            # ---- loads, critical-path order across TWO queues:
            # sync:   ctx_h0 -> wk -> ctx_h1 -> wv -> x2 -> x3 -> wo
            # gpsimd: wq -> x0 -> x1
            # the first scores only need KT (ctx+wk) and QT chunks 0/1
            # (wq+x0/x1, on the parallel queue); V is deferred to group-0
            # extras so it doesn't sit on the critical DMA path.
            # ctx image is [p, jh, kb, 512]: two j-half tiles
            ctxt_sb = [consts.tile([P, KC, 512], BF16, tag=f"ctxt{jh}",
                                   name=f"ctxt{jh}") for jh in range(2)]

            def load_ctx(jh):
                nc.sync.dma_start(
                    out=ctxt_sb[jh],
                    in_=ctxt[:, jh * KC * 512:(jh + 1) * KC * 512]
                    .rearrange("p (kb j) -> p kb j", kb=KC))

            load_ctx(0)
            wk_sb = consts.tile([P, KC, DG], BF16, tag="wk_sb")
            nc.sync.dma_start(
                out=wk_sb, in_=wk[:, :].rearrange("p (kb d) -> p kb d", kb=KC))
            load_ctx(1)
            wv_sb = consts.tile([P, KC, DG], BF16, tag="wv_sb")
            nc.sync.dma_start(
                out=wv_sb, in_=wv[:, :].rearrange("p (kb d) -> p kb d", kb=KC))

            wq_sb = consts.tile([P, KQ, DG], BF16, tag="wq_sb")
            nc.gpsimd.dma_start(
                out=wq_sb, in_=wq[:, :].rearrange("p (kb d) -> p kb d", kb=KQ))
            xq_sb = consts.tile([P, ICN, KQ, 512], BF16, tag="xq_sb")

            def load_x(ich, eng):
                eng.dma_start(
                    out=xq_sb[:, ich],
                    in_=xt[:, ich * KQ * 512:(ich + 1) * KQ * 512]
                    .rearrange("p (kb i) -> p kb i", kb=KQ))

            load_x(0, nc.gpsimd)
            load_x(1, nc.gpsimd)
            load_x(2, nc.sync)
            load_x(3, nc.sync)
            wo_sb = consts.tile([P, TD, E], BF16, tag="wo_sb")
            nc.sync.dma_start(
                out=wo_sb, in_=wo[:, :].rearrange("p (kb e) -> p kb e", kb=TD))

            # ---- projections
            v_sb = [consts.tile([P, HPC, DH + 1], BF16, tag=f"v{jb}",
                                name=f"v{jb}") for jb in range(JBN)]

            def emit_v(jb):
                nc.vector.memset(v_sb[jb][:, :, DH:DH + 1], 1.0)
                ps = pp2.tile([P, DG], F32, tag="pp2", name="vps")
                jh, jo = jb // 4, (jb % 4) * P
                for kb in range(KC):
                    nc.tensor.matmul(
                        ps,
                        lhsT=ctxt_sb[jh][:, kb, jo:jo + P],
                        rhs=wv_sb[:, kb, :],
                        start=(kb == 0), stop=(kb == KC - 1),
                    )
                nc.vector.tensor_copy(
                    v_sb[jb][:, :, 0:DH],
                    ps.rearrange("p (h d) -> p h d", h=HPC),
                )

            kt_sb = [consts.tile([P, J], BF16, tag=f"kt{t}", name=f"kt{t}")
                     for t in range(TD)]

            def emit_kt_nch(t, nch):
                ps = pp2.tile([P, 512], F32, tag="pp2", name="ktps")
                for kb in range(KC):
                    nc.tensor.matmul(
                        ps,
                        lhsT=wk_sb[:, kb, t * P:(t + 1) * P],
                        rhs=ctxt_sb[nch][:, kb, :],
                        start=(kb == 0), stop=(kb == KC - 1),
                    )
                nc.vector.tensor_copy(
                    kt_sb[t][:, nch * 512:(nch + 1) * 512], ps)

            def emit_kt(t):
                for nch in range(2):
                    emit_kt_nch(t, nch)

            # one tile per (t, i-chunk): Tile tracks SBUF deps per tile,
            # so per-chunk tiles let the first scores run before all of x
            # has even arrived
            qt_sb = [[consts.tile([P, 512], BF16, tag=f"qt{t}{ich}",
                                  name=f"qt{t}{ich}") for ich in range(ICN)]
                     for t in range(TD)]

            def emit_qt(ich, t):
                ps = pp2.tile([P, 512], F32, tag="pp2", name="qtps")
                for kb in range(KQ):
                    nc.tensor.matmul(
                        ps,
                        lhsT=wq_sb[:, kb, t * P:(t + 1) * P],
                        rhs=xq_sb[:, ich, kb, :],
                        start=(kb == 0), stop=(kb == KQ - 1),
                    )
                nc.vector.tensor_copy(qt_sb[t][ich], ps)

            # prologue PE work chasing the DMA: KT-nch0 (ctx_h0+wk), then
            # KT-nch1, then the two QT chunks the first scores need
            emit_kt_nch(0, 0)
            emit_kt_nch(0, 1)
            emit_qt(0, 0)
            emit_qt(1, 0)

            o2t_sb = [[consts.tile([P, IH], BF16, tag=f"o2t{half}{t}",
                                   name=f"o2t{half}{t}")
                       for t in range(TD)] for half in range(2)]

            avtile = [None]

            def emit_av_par(half, hp, par, ets, jbs):
                for jb in jbs:
                    for nch in range(2):
                        csl = slice(nch * 512, (nch + 1) * 512)
                        nc.tensor.matmul(
                            avtile[0][:, csl],
                            lhsT=v_sb[jb][:, 2 * hp + par, :],
                            rhs=ets[par][jb][:, csl],
                            start=(jb == 0), stop=(jb == JBN - 1),
                        )

            def emit_norm(half, hp, par, fast=False, cols=None, direct=False):
                h = 2 * hp + par
                av = avtile[0]
                c0, c1 = (0, IH) if cols is None else cols
                cw = c1 - c0
                if direct:
                    # final group: av psum has no next tenant, so the
                    # multiply reads it in place and only the denom row is
                    # staged to SBUF (ScalarE is idle here; DMA can't read
                    # PSUM) — skips the full araw copy
                    rowt = misc.tile([1, IH], F32, tag="rowt", name="rowt")
                    nc.scalar.activation(
                        out=rowt[:, c0:c1], in_=av[DH:DH + 1, c0:c1],
                        func=mybir.ActivationFunctionType.Copy)
                    src_row = rowt[:, c0:c1]
                    mul_src = av[0:DH, c0:c1]
                else:
                    araw = misc.tile([DH + 1, IH], F32, tag="araw",
                                     name="araw")
                    nc.vector.tensor_copy(araw, av)
                    src_row = araw[DH:DH + 1, c0:c1]
                    mul_src = araw[0:DH, c0:c1]
                bidx = half * HPC + h
                # reciprocal is ~7 cyc/elem/lane: transpose the denom row
                # to [128, cw/128] via DRAM so all lanes share the work,
                # then broadcast the recip row back across 64 partitions
                dma_eng = nc.sync if fast else nc.gpsimd
                dma_eng.dma_start(out=brc[bidx:bidx + 1, c0:c1], in_=src_row)
                rcol = misc.tile([P, IH // P], F32, tag="rcol", name="rcol")
                dma_eng.dma_start(
                    out=rcol[:, 0:cw // P],
                    in_=brc[bidx, c0:c1].rearrange("(p t) -> p t", p=P),
                )
                rrec = misc.tile([P, IH // P], F32, tag="rrec", name="rrec")
                nc.vector.reciprocal(rrec[:, 0:cw // P], rcol[:, 0:cw // P])
                dma_eng.dma_start(
                    out=brc2[bidx, c0:c1].rearrange("(p t) -> p t", p=P),
                    in_=rrec[:, 0:cw // P],
                )
                bc = misc.tile([DH, IH], F32, tag="bc", name="bc")
                row = brc2[bidx:bidx + 1, c0:c1]
                dma_eng.dma_start(
                    out=bc[:, 0:cw],
                    in_=bass.AP(tensor=row.tensor, offset=row.offset,
                                ap=[[0, DH]] + row.ap[1:]),
                )
                nc.vector.tensor_mul(
                    o2t_sb[half][hp][par * DH:par * DH + DH, c0:c1],
                    mul_src, bc[:, 0:cw]
                )

            def emit_wo_evac(half, m, pss, tail, split=False):
                # mid-pipeline: both psum evacuations on DVE (ScalarE is
                # exp-saturated); drain/tail: split ACT/DVE so the norm
                # chains' DVE ops aren't queued behind a burst of casts
                ot = outp.tile([P, E], BF16, tag="ot", name="ot")
                r0 = half * IH + m * P
                for nch in range(2):
                    dst = ot[:, nch * 512:(nch + 1) * 512]
                    if (tail or split) and nch == 1:
                        nc.scalar.activation(
                            out=dst, in_=pss[nch],
                            func=mybir.ActivationFunctionType.Copy)
                    else:
                        nc.vector.tensor_copy(dst, pss[nch])
                    if tail:
                        # per-half writes on separate queues: the last
                        # block's output starts moving as soon as each
                        # evacuation lands
                        eng = nc.sync if nch == 0 else nc.gpsimd
                        eng.dma_start(
                            out=out[r0:r0 + P, nch * 512:(nch + 1) * 512],
                            in_=dst)
                if not tail:
                    nc.sync.dma_start(out=out[r0:r0 + P, :], in_=ot)

            def emit_wo_m(half, m, tail=False, deep=False, split=False):
                if deep:
                    big = pp.tile([P, IH], F32, tag="pp", name="wobig")
                    pss = [big[:, 0:512], big[:, 512:1024]]
                else:
                    pss = [pp2.tile([P, 512], F32, tag="pp2",
                                    name=f"wopp{n}") for n in range(2)]
                for t in range(TD):
                    for nch in range(2):
                        nc.tensor.matmul(
                            pss[nch],
                            lhsT=o2t_sb[half][t][:, m * P:(m + 1) * P],
                            rhs=wo_sb[:, t, nch * 512:(nch + 1) * 512],
                            start=(t == 0), stop=(t == TD - 1),
                        )
                emit_wo_evac(half, m, pss, tail, split)

            def emit_wo_part(half, m, pss, t, start, stop):
                for nch in range(2):
                    nc.tensor.matmul(
                        pss[nch],
                        lhsT=o2t_sb[half][t][:, m * P:(m + 1) * P],
                        rhs=wo_sb[:, t, nch * 512:(nch + 1) * 512],
                        start=start, stop=stop,
                    )

            # ---- attention schedule: per-jb fine interleave so the
            # in-order PE stream never bursts long enough to starve ACT.
            # extras = deferred PE work (QT chunks, KT t1, Wo m-blocks)
            # popped between the scores/exp/AV groups.
            pending = None
            for k, (half, hp) in enumerate([(0, 0), (0, 1), (1, 0), (1, 1)]):
                extras = []
                if k == 0:
                    # ALL V projections must be emitted here: the flat AV
                    # queue below reads v_sb[jb] from k==1 on, and Tile
                    # builds deps from emission order
                    extras = ([(lambda jb=jb: emit_v(jb))
                               for jb in range(JBN)]
                              + [lambda: emit_kt(1),
                                 lambda: emit_qt(0, 1), lambda: emit_qt(1, 1),
                                 lambda: emit_qt(2, 0), lambda: emit_qt(3, 0)])
                elif k == 1:
                    extras = [lambda: emit_qt(2, 1), lambda: emit_qt(3, 1)]
                elif k == 3:
                    extras = [(lambda m=m: emit_wo_m(0, m))
                              for m in range(4)]
                prev = pending
                if prev is not None:
                    avtile[0] = avpool.tile([DH + 1, IH], F32, tag="av",
                                            name="av")
                avq = []
                if prev is not None:
                    avq = ([(0, jb) for jb in range(JBN)] + ["norm0"]
                           + [(1, jb) for jb in range(JBN)] + ["norm1"])

                def pop_av():
                    item = avq.pop(0)
                    if item == "norm0":
                        emit_norm(prev[0], prev[1], 0)
                        avtile[0] = avpool.tile([DH + 1, IH], F32, tag="av",
                                                name="av")
                    elif item == "norm1":
                        emit_norm(prev[0], prev[1], 1)
                    else:
                        emit_av_par(prev[0], prev[1], item[0], prev[2],
                                    [item[1]])

                t = hp
                ets = [[None] * JBN, [None] * JBN]
                for jb in range(JBN):
                    scs = []
                    for par in range(2):
                        prow = par * DH
                        sc = pp.tile([P, IH], F32, tag="pp", name=f"sc{par}")
                        for nch in range(2):
                            nc.tensor.matmul(
                                sc[:, nch * 512:(nch + 1) * 512],
                                lhsT=kt_sb[t][prow:prow + DH,
                                              jb * P:(jb + 1) * P],
                                rhs=qt_sb[t][half * 2 + nch][prow:prow + DH, :],
                                start=True, stop=True,
                            )
                        scs.append(sc)
                    for par in range(2):
                        et = expp.tile([P, IH], BF16, tag="et",
                                       name=f"et{par}")
                        nc.scalar.activation(
                            out=et, in_=scs[par],
                            func=mybir.ActivationFunctionType.Exp,
                            scale=0.125,
                        )
                        ets[par][jb] = et
                    for _ in range(3):
                        if avq:
                            pop_av()
                    for _ in range(2):
                        if extras:
                            extras.pop(0)()
                while avq:
                    pop_av()
                while extras:
                    extras.pop(0)()
                pending = (half, hp, ets)

            # ---- drain the last group (1,1).  norm0 is emitted right
            # after par0's AV so its DVE ops (araw/recip/mul) sit at the
            # head of the vector queue — par1's AV + Wo(0)'s second wave
            # (evacs split DVE/ACT) cover its DMA chain.  During par1's
            # norm window the PE runs the t=0 partials of Wo(1, m=0..3)
            # (their lhsT, o2t[1][t=0] from group (1,0), is already
            # final), then finishes them and the rest of Wo(1).
            half, hp, ets = pending
            avtile[0] = avpool.tile([DH + 1, IH], F32, tag="av", name="av")
            for jb in range(JBN):
                emit_av_par(half, hp, 0, ets, [jb])
            emit_norm(half, hp, 0, fast=False)
            avtile[0] = avpool.tile([DH + 1, IH], F32, tag="av", name="av")
            for jb in range(JBN):
                emit_av_par(half, hp, 1, ets, [jb])
                if jb < 2:
                    emit_wo_m(0, 4 + jb, split=True)
            emit_norm(half, hp, 1, fast=True)
            # fill the norm1 DMA window: the last two Wo(0) blocks have no
            # norm1 dependency
            emit_wo_m(0, 6, split=True)
            emit_wo_m(0, 7, split=True)
            # t0-partials: m0/m1 in the two pp bufs, m2 in the avpool buf,
            # m3 split across the two pp2 bufs (8 banks total, all free)
            wop = []
            for m in range(2):
                big = pp.tile([P, IH], F32, tag="pp", name=f"wop{m}")
                wop.append([big[:, 0:512], big[:, 512:1024]])
            big = avpool.tile([P, IH], F32, tag="av", name="wop2")
            wop.append([big[:, 0:512], big[:, 512:1024]])
            wop.append([pp2.tile([P, 512], F32, tag="pp2",
                                 name=f"wop3{n}") for n in range(2)])
            for m in range(4):
                emit_wo_part(1, m, wop[m], t=0, start=True, stop=False)
            for m in range(4):
                emit_wo_part(1, m, wop[m], t=1, start=False, stop=True)
                emit_wo_evac(1, m, wop[m], tail=True)
            for m in range(4, 8):
                emit_wo_m(1, m, tail=True, deep=(m % 2 == 0))

    nc.compile()
    return nc


_NC_CACHE = None


def _get_nc():
    global _NC_CACHE
    if _NC_CACHE is None:
        _NC_CACHE = _build()
    return _NC_CACHE


def _sbuf_image(a):
    """[KB*128, R] row-major -> [128, KB*R]: partition p holds the
    concatenation of rows {kb*128+p} (one contiguous run per partition)."""
    kb = a.shape[0] // P
    return np.ascontiguousarray(
        a.reshape(kb, P, a.shape[1]).transpose(1, 0, 2).reshape(P, -1)
    ).astype(ml_dtypes.bfloat16)


def _x_image(xtb):
    """x^T [1024, 2048] -> per partition: [ich, kb, 512] contiguous."""
    r = xtb.reshape(KQ, P, ICN, 512).transpose(1, 2, 0, 3)
    return np.ascontiguousarray(r.reshape(P, -1)).astype(ml_dtypes.bfloat16)


def _ctx_image(ctb):
    """ctx^T [768, 1024] -> per partition: [jh, kb, 512] contiguous."""
    r = ctb.reshape(KC, P, 2, 512).transpose(1, 2, 0, 3)
    return np.ascontiguousarray(r.reshape(P, -1)).astype(ml_dtypes.bfloat16)


def _make_in_maps(x, context, Wq, Wk, Wv, Wo):
    in_maps = []
    for c in range(N_CORES):
        b, hg = c // 2, c % 2
        sl = slice(hg * DG, (hg + 1) * DG)
        in_maps.append({
            "xt": _x_image(x[b].T),
            "ctxt": _ctx_image(context[b].T),
            "wq": _sbuf_image(Wq[:, sl]),
            "wk": _sbuf_image(Wk[:, sl]),
            "wv": _sbuf_image(Wv[:, sl]),
            "wo": _sbuf_image(Wo[sl, :]),
        })
    return in_maps


def _run(inputs, trace=False):
    x = np.asarray(inputs["x"], dtype=np.float32)
    context = np.asarray(inputs["context"], dtype=np.float32)
    Wq = np.asarray(inputs["Wq"], dtype=np.float32)
    Wk = np.asarray(inputs["Wk"], dtype=np.float32)
    Wv = np.asarray(inputs["Wv"], dtype=np.float32)
    Wo = np.asarray(inputs["Wo"], dtype=np.float32)
    bo = np.asarray(inputs["bo"], dtype=np.float32)

    res = run_bass_kernel_spmd(
        _get_nc(), _make_in_maps(x, context, Wq, Wk, Wv, Wo),
        core_ids=list(range(N_CORES)), trace=trace,
    )
    parts = [np.asarray(r["out"], dtype=np.float32) for r in res.results]
    outv = np.stack([parts[2 * b] + parts[2 * b + 1] + bo for b in range(B)])
    return outv.astype(np.float32), res


def kernel(**inputs) -> np.ndarray:
    outv, _ = _run(inputs, trace=False)
    return outv


# revision 26
# speedup vs baseline: 1.0023x; 1.0023x over previous
"""Cross-attention kernel for 8 TRN2 NeuronCores (Bass/Tile).

Reference (fp32):
    q = x @ Wq; k = ctx @ Wk; v = ctx @ Wv        (8 heads, d=64)
    sim = q k^T * d^-0.5 ; attn = softmax(sim)
    out = (attn v) @ Wo + bo

Sharding (data-parallel, no FLOP duplication): core c -> batch c//2,
head-group c%2 (4 heads).  Each core computes a partial [2048, 1024]
output; the host sums the two partials per batch and adds bo.

Per-core dataflow (bf16 matmul operands, fp32 accumulation):
  - host pre-shuffles every input into its exact SBUF image
    ([128, fat-contiguous-run] per partition) for full-rate DMA; x is
    i-chunk-major, ctx is j-half-major (so V projections can start on
    the first half while the second streams in)
  - input DMA is split across the sync and gpsimd queues in critical-
    path order: {wv, ctx, wk, x2, x3, wo} on sync, {wq, x0, x1} on
    gpsimd; V -> KT -> QT projections chase the DMA so the PE is doing
    useful work from ~7us (a short junk-matmul burst first trips the
    HAM clock-gate out of its cold 0.65-1.2 GHz p-state)
  - QT[d,i] = Wq^T x^T ; KT[d,j] = Wk^T ctx^T ; V[j,d] = ctx Wv, with a
    ones column appended per head ([V_h | 1], memset on-device)
  - simT[j,i] = KT_h-slice @ QT_h (K=64; head pairs land on PE row
    groups 0-63/64-127), fp32 PSUM
  - expT = exp(0.125 simT) on ScalarE (unsafe softmax, |sim|*0.125 < 4)
  - av = [V_h|1]^T @ expT accumulated over j -> [65, i] fp32 PSUM;
    row 64 is the softmax denominator
  - denominator: transpose to [128, 8] via DRAM (reciprocal is
    ~7 cyc/elem/lane), reciprocal, broadcast over 64 partitions via a
    DRAM round-trip; out2T = av[0:64] * recip  -> exactly the lhsT
    layout the Wo matmul needs
  - out = out2T^T @ Wo: half0 mid-attention (PE slack); half1's m=0..3
    t0-partials run during the last norm's DMA-latency window (their
    lhsT half is ready one group earlier), so the PE never idles long
    enough for HAM to re-throttle; psum evacuation is DVE mid-pipeline
    (ScalarE is exp-saturated) and ACT/DVE alternating in the tail;
    output staged bf16 (host accumulates in fp32)
The kernel is PE-stream-bound (~221k matmul columns/core at 2.4 GHz);
ScalarE (64 exps over 8.4M elements) runs just under it.
"""

import numpy as np
import ml_dtypes

import concourse.bass as bass
import concourse.tile as tile
from concourse import bacc, mybir
from concourse.bass_utils import run_bass_kernel_spmd

B = 4
I = 2048
J = 1024
FQ = 1024
FC = 768
DH = 64
HPC = 4
DG = HPC * DH      # 256
E = 1024
P = 128
N_CORES = 8
IH = I // 2        # 1024

F32 = mybir.dt.float32
BF16 = mybir.dt.bfloat16

KQ = FQ // P       # 8
KC = FC // P       # 6
TD = DG // P       # 2
JBN = J // P       # 8
ICN = 4            # x i-chunks


def _build():
    nc = bacc.Bacc()
    xt = nc.declare_dram_parameter("xt", [P, KQ * I], BF16, isOutput=False)
    ctxt = nc.declare_dram_parameter("ctxt", [P, KC * J], BF16, isOutput=False)
    wq = nc.declare_dram_parameter("wq", [P, KQ * DG], BF16, isOutput=False)
    wk = nc.declare_dram_parameter("wk", [P, KC * DG], BF16, isOutput=False)
    wv = nc.declare_dram_parameter("wv", [P, KC * DG], BF16, isOutput=False)
    wo = nc.declare_dram_parameter("wo", [P, TD * E], BF16, isOutput=False)
    out = nc.declare_dram_parameter("out", [I, E], BF16, isOutput=True)
    brc = nc.dram_tensor("brc", [2 * HPC, IH], F32)
    brc2 = nc.dram_tensor("brc2", [2 * HPC, IH], F32)

    with tile.TileContext(nc) as tc:
        with (
            tc.tile_pool(name="consts", bufs=1) as consts,
            tc.tile_pool(name="expp", bufs=40) as expp,
            tc.tile_pool(name="misc", bufs=3) as misc,
            tc.tile_pool(name="outp", bufs=3) as outp,
            tc.tile_pool(name="pp", bufs=2, space="PSUM") as pp,
            tc.tile_pool(name="pp2", bufs=2, space="PSUM") as pp2,
            tc.tile_pool(name="avp", bufs=1, space="PSUM") as avpool,
        ):
            # ---- PE warm-up: junk matmuls trip the HAM clock-gate (cold
            # PE runs at a low p-state) while the first loads stream in
            junk = consts.tile([P, P], BF16, tag="junk")
            nc.vector.memset(junk, 0.0)
            jps = pp2.tile([P, 512], F32, tag="pp2", name="jps")
            for w in range(48):
                nc.tensor.matmul(jps[:, :P], lhsT=junk, rhs=junk,
                                 start=True, stop=True)

            # ---- loads, critical-path order across TWO queues:
            # sync:   ctx_h0 -> wk -> ctx_h1 -> wv -> x2 -> x3 -> wo
            # gpsimd: wq -> x0 -> x1
            # the first scores only need KT (ctx+wk) and QT chunks 0/1
            # (wq+x0/x1, on the parallel queue); V is deferred to group-0
            # extras so it doesn't sit on the critical DMA path.
            # ctx image is [p, jh, kb, 512]: two j-half tiles
            ctxt_sb = [consts.tile([P, KC, 512], BF16, tag=f"ctxt{jh}",
                                   name=f"ctxt{jh}") for jh in range(2)]

            def load_ctx(jh):
                nc.sync.dma_start(
                    out=ctxt_sb[jh],
                    in_=ctxt[:, jh * KC * 512:(jh + 1) * KC * 512]
                    .rearrange("p (kb j) -> p kb j", kb=KC))

            load_ctx(0)
            wk_sb = consts.tile([P, KC, DG], BF16, tag="wk_sb")
            nc.sync.dma_start(
                out=wk_sb, in_=wk[:, :].rearrange("p (kb d) -> p kb d", kb=KC))
            load_ctx(1)
            wv_sb = consts.tile([P, KC, DG], BF16, tag="wv_sb")
            nc.sync.dma_start(
                out=wv_sb, in_=wv[:, :].rearrange("p (kb d) -> p kb d", kb=KC))

            wq_sb = consts.tile([P, KQ, DG], BF16, tag="wq_sb")
            nc.gpsimd.dma_start(
                out=wq_sb, in_=wq[:, :].rearrange("p (kb d) -> p kb d", kb=KQ))
            xq_sb = consts.tile([P, ICN, KQ, 512], BF16, tag="xq_sb")

            def load_x(ich, eng):
                eng.dma_start(
                    out=xq_sb[:, ich],
                    in_=xt[:, ich * KQ * 512:(ich + 1) * KQ * 512]
                    .rearrange("p (kb i) -> p kb i", kb=KQ))

            load_x(0, nc.gpsimd)
            load_x(1, nc.gpsimd)
            load_x(2, nc.sync)
            load_x(3, nc.sync)
            wo_sb = consts.tile([P, TD, E], BF16, tag="wo_sb")
            nc.sync.dma_start(
                out=wo_sb, in_=wo[:, :].rearrange("p (kb e) -> p kb e", kb=TD))

            # ---- projections
            v_sb = [consts.tile([P, HPC, DH + 1], BF16, tag=f"v{jb}",
                                name=f"v{jb}") for jb in range(JBN)]

            def emit_v(jb):
                nc.vector.memset(v_sb[jb][:, :, DH:DH + 1], 1.0)
                ps = pp2.tile([P, DG], F32, tag="pp2", name="vps")
                jh, jo = jb // 4, (jb % 4) * P
                for kb in range(KC):
                    nc.tensor.matmul(
                        ps,
                        lhsT=ctxt_sb[jh][:, kb, jo:jo + P],
                        rhs=wv_sb[:, kb, :],
                        start=(kb == 0), stop=(kb == KC - 1),
                    )
                nc.vector.tensor_copy(
                    v_sb[jb][:, :, 0:DH],
                    ps.rearrange("p (h d) -> p h d", h=HPC),
                )

            kt_sb = [consts.tile([P, J], BF16, tag=f"kt{t}", name=f"kt{t}")
                     for t in range(TD)]

            def emit_kt_nch(t, nch):
                ps = pp2.tile([P, 512], F32, tag="pp2", name="ktps")
                for kb in range(KC):
                    nc.tensor.matmul(
                        ps,
                        lhsT=wk_sb[:, kb, t * P:(t + 1) * P],
                        rhs=ctxt_sb[nch][:, kb, :],
                        start=(kb == 0), stop=(kb == KC - 1),
                    )
                nc.vector.tensor_copy(
                    kt_sb[t][:, nch * 512:(nch + 1) * 512], ps)

            def emit_kt(t):
                for nch in range(2):
                    emit_kt_nch(t, nch)

            # one tile per (t, i-chunk): Tile tracks SBUF deps per tile,
            # so per-chunk tiles let the first scores run before all of x
            # has even arrived
            qt_sb = [[consts.tile([P, 512], BF16, tag=f"qt{t}{ich}",
                                  name=f"qt{t}{ich}") for ich in range(ICN)]
                     for t in range(TD)]

            def emit_qt(ich, t):
                ps = pp2.tile([P, 512], F32, tag="pp2", name="qtps")
                for kb in range(KQ):
                    nc.tensor.matmul(
                        ps,
                        lhsT=wq_sb[:, kb, t * P:(t + 1) * P],
                        rhs=xq_sb[:, ich, kb, :],
                        start=(kb == 0), stop=(kb == KQ - 1),
                    )
                nc.vector.tensor_copy(qt_sb[t][ich], ps)

            # prologue PE work chasing the DMA: KT-nch0 (ctx_h0+wk), then
            # KT-nch1, then the two QT chunks the first scores need
            emit_kt_nch(0, 0)
            emit_kt_nch(0, 1)
            emit_qt(0, 0)
            emit_qt(1, 0)

            o2t_sb = [[consts.tile([P, IH], BF16, tag=f"o2t{half}{t}",
                                   name=f"o2t{half}{t}")
                       for t in range(TD)] for half in range(2)]

            avtile = [None]

            def emit_av_par(half, hp, par, ets, jbs):
                for jb in jbs:
                    for nch in range(2):
                        csl = slice(nch * 512, (nch + 1) * 512)
                        nc.tensor.matmul(
                            avtile[0][:, csl],
                            lhsT=v_sb[jb][:, 2 * hp + par, :],
                            rhs=ets[par][jb][:, csl],
                            start=(jb == 0), stop=(jb == JBN - 1),
                        )

            def emit_norm(half, hp, par, fast=False, cols=None, direct=False):
                h = 2 * hp + par
                av = avtile[0]
                c0, c1 = (0, IH) if cols is None else cols
                cw = c1 - c0
                if direct:
                    # final group: av psum has no next tenant, so the
                    # multiply reads it in place and only the denom row is
                    # staged to SBUF (ScalarE is idle here; DMA can't read
                    # PSUM) — skips the full araw copy
                    rowt = misc.tile([1, IH], F32, tag="rowt", name="rowt")
                    nc.scalar.activation(
                        out=rowt[:, c0:c1], in_=av[DH:DH + 1, c0:c1],
                        func=mybir.ActivationFunctionType.Copy)
                    src_row = rowt[:, c0:c1]
                    mul_src = av[0:DH, c0:c1]
                else:
                    araw = misc.tile([DH + 1, IH], F32, tag="araw",
                                     name="araw")
                    nc.vector.tensor_copy(araw, av)
                    src_row = araw[DH:DH + 1, c0:c1]
                    mul_src = araw[0:DH, c0:c1]
                bidx = half * HPC + h
                # reciprocal is ~7 cyc/elem/lane: transpose the denom row
                # to [128, cw/128] via DRAM so all lanes share the work,
                # then broadcast the recip row back across 64 partitions
                dma_eng = nc.sync if fast else nc.gpsimd
                dma_eng.dma_start(out=brc[bidx:bidx + 1, c0:c1], in_=src_row)
                rcol = misc.tile([P, IH // P], F32, tag="rcol", name="rcol")
                dma_eng.dma_start(
                    out=rcol[:, 0:cw // P],
                    in_=brc[bidx, c0:c1].rearrange("(p t) -> p t", p=P),
                )
                rrec = misc.tile([P, IH // P], F32, tag="rrec", name="rrec")
                nc.vector.reciprocal(rrec[:, 0:cw // P], rcol[:, 0:cw // P])
                dma_eng.dma_start(
                    out=brc2[bidx, c0:c1].rearrange("(p t) -> p t", p=P),
                    in_=rrec[:, 0:cw // P],
                )
                bc = misc.tile([DH, IH], F32, tag="bc", name="bc")
                row = brc2[bidx:bidx + 1, c0:c1]
                dma_eng.dma_start(
                    out=bc[:, 0:cw],
                    in_=bass.AP(tensor=row.tensor, offset=row.offset,
                                ap=[[0, DH]] + row.ap[1:]),
                )
                nc.vector.tensor_mul(
                    o2t_sb[half][hp][par * DH:par * DH + DH, c0:c1],
                    mul_src, bc[:, 0:cw]
                )

            def emit_wo_evac(half, m, pss, tail, split=False):
                # mid-pipeline: both psum evacuations on DVE (ScalarE is
                # exp-saturated); drain/tail: split ACT/DVE so the norm
                # chains' DVE ops aren't queued behind a burst of casts
                ot = outp.tile([P, E], BF16, tag="ot", name="ot")
                for nch in range(2):
                    dst = ot[:, nch * 512:(nch + 1) * 512]
                    if (tail or split) and nch == 1:
                        nc.scalar.activation(
                            out=dst, in_=pss[nch],
                            func=mybir.ActivationFunctionType.Copy)
                    else:
                        nc.vector.tensor_copy(dst, pss[nch])
                r0 = half * IH + m * P
                eng = nc.gpsimd if (tail and m % 2 == 0) else nc.sync
                eng.dma_start(out=out[r0:r0 + P, :], in_=ot)

            def emit_wo_m(half, m, tail=False, deep=False, split=False):
                if deep:
                    big = pp.tile([P, IH], F32, tag="pp", name="wobig")
                    pss = [big[:, 0:512], big[:, 512:1024]]
                else:
                    pss = [pp2.tile([P, 512], F32, tag="pp2",
                                    name=f"wopp{n}") for n in range(2)]
                for t in range(TD):
                    for nch in range(2):
                        nc.tensor.matmul(
                            pss[nch],
                            lhsT=o2t_sb[half][t][:, m * P:(m + 1) * P],
                            rhs=wo_sb[:, t, nch * 512:(nch + 1) * 512],
                            start=(t == 0), stop=(t == TD - 1),
                        )
                emit_wo_evac(half, m, pss, tail, split)

            def emit_wo_part(half, m, pss, t, start, stop):
                for nch in range(2):
                    nc.tensor.matmul(
                        pss[nch],
                        lhsT=o2t_sb[half][t][:, m * P:(m + 1) * P],
                        rhs=wo_sb[:, t, nch * 512:(nch + 1) * 512],
                        start=start, stop=stop,
                    )

            # ---- attention schedule: per-jb fine interleave so the
            # in-order PE stream never bursts long enough to starve ACT.
            # extras = deferred PE work (QT chunks, KT t1, Wo m-blocks)
            # popped between the scores/exp/AV groups.
            pending = None
            for k, (half, hp) in enumerate([(0, 0), (0, 1), (1, 0), (1, 1)]):
                extras = []
                if k == 0:
                    # ALL V projections must be emitted here: the flat AV
                    # queue below reads v_sb[jb] from k==1 on, and Tile
                    # builds deps from emission order
                    extras = ([(lambda jb=jb: emit_v(jb))
                               for jb in range(JBN)]
                              + [lambda: emit_kt(1),
                                 lambda: emit_qt(0, 1), lambda: emit_qt(1, 1),
                                 lambda: emit_qt(2, 0), lambda: emit_qt(3, 0)])
                elif k == 1:
                    extras = [lambda: emit_qt(2, 1), lambda: emit_qt(3, 1)]
                elif k == 3:
                    extras = [(lambda m=m: emit_wo_m(0, m))
                              for m in range(4)]
                prev = pending
                if prev is not None:
                    avtile[0] = avpool.tile([DH + 1, IH], F32, tag="av",
                                            name="av")
                avq = []
                if prev is not None:
                    avq = ([(0, jb) for jb in range(JBN)] + ["norm0"]
                           + [(1, jb) for jb in range(JBN)] + ["norm1"])

                def pop_av():
                    item = avq.pop(0)
                    if item == "norm0":
                        emit_norm(prev[0], prev[1], 0)
                        avtile[0] = avpool.tile([DH + 1, IH], F32, tag="av",
                                                name="av")
                    elif item == "norm1":
                        emit_norm(prev[0], prev[1], 1)
                    else:
                        emit_av_par(prev[0], prev[1], item[0], prev[2],
                                    [item[1]])

                t = hp
                ets = [[None] * JBN, [None] * JBN]
                for jb in range(JBN):
                    scs = []
                    for par in range(2):
                        prow = par * DH
                        sc = pp.tile([P, IH], F32, tag="pp", name=f"sc{par}")
                        for nch in range(2):
                            nc.tensor.matmul(
                                sc[:, nch * 512:(nch + 1) * 512],
                                lhsT=kt_sb[t][prow:prow + DH,
                                              jb * P:(jb + 1) * P],
                                rhs=qt_sb[t][half * 2 + nch][prow:prow + DH, :],
                                start=True, stop=True,
                            )
                        scs.append(sc)
                    for par in range(2):
                        et = expp.tile([P, IH], BF16, tag="et",
                                       name=f"et{par}")
                        nc.scalar.activation(
                            out=et, in_=scs[par],
                            func=mybir.ActivationFunctionType.Exp,
                            scale=0.125,
                        )
                        ets[par][jb] = et
                    for _ in range(3):
                        if avq:
                            pop_av()
                    for _ in range(2):
                        if extras:
                            extras.pop(0)()
                while avq:
                    pop_av()
                while extras:
                    extras.pop(0)()
                pending = (half, hp, ets)

            # ---- drain the last group (1,1).  norm0 is emitted right
            # after par0's AV so its DVE ops (araw/recip/mul) sit at the
            # head of the vector queue — par1's AV + Wo(0)'s second wave
            # (evacs split DVE/ACT) cover its DMA chain.  During par1's
            # norm window the PE runs the t=0 partials of Wo(1, m=0..3)
            # (their lhsT, o2t[1][t=0] from group (1,0), is already
            # final), then finishes them and the rest of Wo(1).
            half, hp, ets = pending
            avtile[0] = avpool.tile([DH + 1, IH], F32, tag="av", name="av")
            for jb in range(JBN):
                emit_av_par(half, hp, 0, ets, [jb])
            emit_norm(half, hp, 0, fast=False)
            avtile[0] = avpool.tile([DH + 1, IH], F32, tag="av", name="av")
            for jb in range(JBN):
                emit_av_par(half, hp, 1, ets, [jb])
                if jb < 4:
                    emit_wo_m(0, 4 + jb, split=True)
            emit_norm(half, hp, 1, fast=True)
            # t0-partials: m0/m1 in the two pp bufs, m2 in the avpool buf,
            # m3 split across the two pp2 bufs (8 banks total, all free)
            wop = []
            for m in range(2):
                big = pp.tile([P, IH], F32, tag="pp", name=f"wop{m}")
                wop.append([big[:, 0:512], big[:, 512:1024]])
            big = avpool.tile([P, IH], F32, tag="av", name="wop2")
            wop.append([big[:, 0:512], big[:, 512:1024]])
            wop.append([pp2.tile([P, 512], F32, tag="pp2",
                                 name=f"wop3{n}") for n in range(2)])
            for m in range(4):
                emit_wo_part(1, m, wop[m], t=0, start=True, stop=False)
            for m in range(4):
                emit_wo_part(1, m, wop[m], t=1, start=False, stop=True)
                emit_wo_evac(1, m, wop[m], tail=True)
            for m in range(4, 8):
                emit_wo_m(1, m, tail=True, deep=(m % 2 == 0))

    nc.compile()
    return nc


_NC_CACHE = None


def _get_nc():
    global _NC_CACHE
    if _NC_CACHE is None:
        _NC_CACHE = _build()
    return _NC_CACHE


def _sbuf_image(a):
    """[KB*128, R] row-major -> [128, KB*R]: partition p holds the
    concatenation of rows {kb*128+p} (one contiguous run per partition)."""
    kb = a.shape[0] // P
    return np.ascontiguousarray(
        a.reshape(kb, P, a.shape[1]).transpose(1, 0, 2).reshape(P, -1)
    ).astype(ml_dtypes.bfloat16)


def _x_image(xtb):
    """x^T [1024, 2048] -> per partition: [ich, kb, 512] contiguous."""
    r = xtb.reshape(KQ, P, ICN, 512).transpose(1, 2, 0, 3)
    return np.ascontiguousarray(r.reshape(P, -1)).astype(ml_dtypes.bfloat16)


def _ctx_image(ctb):
    """ctx^T [768, 1024] -> per partition: [jh, kb, 512] contiguous."""
    r = ctb.reshape(KC, P, 2, 512).transpose(1, 2, 0, 3)
    return np.ascontiguousarray(r.reshape(P, -1)).astype(ml_dtypes.bfloat16)


def _make_in_maps(x, context, Wq, Wk, Wv, Wo):
    in_maps = []
    for c in range(N_CORES):
        b, hg = c // 2, c % 2
        sl = slice(hg * DG, (hg + 1) * DG)
        in_maps.append({
            "xt": _x_image(x[b].T),
            "ctxt": _ctx_image(context[b].T),
            "wq": _sbuf_image(Wq[:, sl]),
            "wk": _sbuf_image(Wk[:, sl]),
            "wv": _sbuf_image(Wv[:, sl]),
            "wo": _sbuf_image(Wo[sl, :]),
        })
    return in_maps


def _run(inputs, trace=False):
    x = np.asarray(inputs["x"], dtype=np.float32)
    context = np.asarray(inputs["context"], dtype=np.float32)
    Wq = np.asarray(inputs["Wq"], dtype=np.float32)
    Wk = np.asarray(inputs["Wk"], dtype=np.float32)
    Wv = np.asarray(inputs["Wv"], dtype=np.float32)
    Wo = np.asarray(inputs["Wo"], dtype=np.float32)
    bo = np.asarray(inputs["bo"], dtype=np.float32)

    res = run_bass_kernel_spmd(
        _get_nc(), _make_in_maps(x, context, Wq, Wk, Wv, Wo),
        core_ids=list(range(N_CORES)), trace=trace,
    )
    parts = [np.asarray(r["out"], dtype=np.float32) for r in res.results]
    outv = np.stack([parts[2 * b] + parts[2 * b + 1] + bo for b in range(B)])
    return outv.astype(np.float32), res


def kernel(**inputs) -> np.ndarray:
    outv, _ = _run(inputs, trace=False)
    return outv
